# revision 16
# baseline (speedup 1.0000x reference)
"""Trainium2 Bass kernel for nn_DetectionLoss (B=8, N=131072, C=21, M=32).

Strategy (data-parallel over batch, one image per NeuronCore):
  Device (per core, SPMD over 8 cores):
    - classification side: exp -> group-sum over C=21 -> log  => logS per anchor
    - detection side: for each of M=32 targets, compute intersection i with all
      anchors via fused tensor_scalar / scalar_tensor_tensor ops; compare the
      iou-equivalent ratio r = i/(areaA+areaB) in LOG domain
      (ln(i) - ln(areaA+areaB), ACT engine Ln; ACT Reciprocal is banned).
      Running argmax over m is a float MAX over bitcast(log-ratio) with the
      target index packed into the 5 low mantissa bits (log-ratio < 0 always;
      or-ing index bits grows the magnitude of a negative float, so ties break
      toward the smallest m, matching jnp.argmax).
      iou >= 0.5  <=>  r >= 1/3 ;  iou < 0.3  <=>  r < 3/13 (monotone map).
    - work is split across DVE / ACT / GPSIMD to balance engine occupancy.
    - outputs: per-anchor int8 encoding (argmax idx | pos<<5 | neg<<6) and
      per-partition partial sums (npos, nneg, sum pos*logS, sum neg*logS,
      sum neg*cls0).
  Host: O(npos)-sized gathers for the matched-class CE term and the
  smooth-L1 regression term, then the reference's scalar combine.
"""

import sys
import numpy as np

sys.path.insert(0, "/opt/trn_rl_repo")

import concourse.bass as bass  # noqa: E402
import concourse.tile as tile  # noqa: E402
from concourse import bacc, mybir  # noqa: E402
from concourse.bass_utils import run_bass_kernel_spmd  # noqa: E402

F32 = mybir.dt.float32
I32 = mybir.dt.int32
I8 = mybir.dt.int8
ALU = mybir.AluOpType
AF = mybir.ActivationFunctionType
AX = mybir.AxisListType

B, N, C, M = 8, 131072, 21, 32
P = 128
FTOT = N // P          # 1024 anchors per partition
FC = 128               # cls-stage tile free size
NTC = FTOT // FC       # cls tiles

THR_POS = float(np.float32(np.log(1.0 / 3.0)))    # iou>=0.5  <=> logratio >= ln(1/3)
THR_NEG = float(np.float32(np.log(3.0 / 13.0)))   # iou<0.3   <=> logratio <  ln(3/13)
EPS = 1e-10

# NOTE: the Pool/GpSimd engine rejects TensorTensor/TensorScalar on this ISA
# (walrus NCC_IXCG966), so all elementwise work lives on DVE + ACT.

_CACHE = {}


def _patch_act_tables():
    """Force exp+ln onto the single combined table set (kills table thrash)."""
    from concourse import hw_specs

    orig = hw_specs.get_activation_tables
    if getattr(bacc.get_activation_tables, "_patched", False):
        return

    def patched(arch):
        t = orig(arch)
        exp = mybir.ActivationFunctionType.Exp
        ln = mybir.ActivationFunctionType.Ln
        for name, fns in t.items():
            if name != "natural_log_exp_and_others":
                fns.discard(exp)
                fns.discard(ln)
        return t

    patched._patched = True
    bacc.get_activation_tables = patched


def _build_program():
    _patch_act_tables()
    nc = bacc.Bacc("TRN2", target_bir_lowering=False, debug=False)

    cls_d = nc.dram_tensor("cls", [N, C], F32, kind="ExternalInput").ap()
    reg_d = nc.dram_tensor("reg", [N, 4], F32, kind="ExternalInput").ap()
    tcon_d = nc.dram_tensor("tcon", [P, 7 * M], F32, kind="ExternalInput").ap()
    acc_d = nc.dram_tensor("acc", [P, 8], F32, kind="ExternalOutput").ap()
    enc_d = nc.dram_tensor("enc", [P, FTOT], I8, kind="ExternalOutput").ap()

    cls3 = cls_d.rearrange("(p f) c -> p f c", p=P)   # [128, 1024, 21]
    reg3 = reg_d.rearrange("(p f) c -> p f c", p=P)   # [128, 1024, 4]

    with tile.TileContext(nc) as tc:
        with (
            tc.tile_pool(name="const", bufs=1) as constp,
            tc.tile_pool(name="rows", bufs=1) as rows,
            tc.tile_pool(name="clsbuf", bufs=3) as clsbuf,
            tc.tile_pool(name="mtmp", bufs=2) as mtmp,
        ):
            tcon = constp.tile([P, 7 * M], F32)
            nc.sync.dma_start(tcon[:], tcon_d[:])

            def tcol(c, m):
                # blocks: 0=b1x 1=b1y 2=b2x 3=b2y 4=area_b 5=-b1y 6=-b1x
                return tcon[:, c * M + m : c * M + m + 1]

            # persistent full-row tensors
            regt = rows.tile([P, FTOT * 4], F32)
            ax1 = rows.tile([P, FTOT], F32)
            ay1 = rows.tile([P, FTOT], F32)
            ax2 = rows.tile([P, FTOT], F32)
            ay2 = rows.tile([P, FTOT], F32)
            areaA = rows.tile([P, FTOT], F32)
            state = rows.tile([P, FTOT], F32)
            sumexp = rows.tile([P, FTOT], F32)
            logS = rows.tile([P, FTOT], F32)
            cls0 = rows.tile([P, FTOT], F32)
            posm = rows.tile([P, FTOT], F32)
            negm = rows.tile([P, FTOT], F32)
            idxf = rows.tile([P, FTOT], F32)
            encf = rows.tile([P, FTOT], F32)
            dummy = rows.tile([P, FTOT], F32)
            gbits = rows.tile([P, FTOT], I32)
            idx32 = rows.tile([P, FTOT], I32)
            enc8 = rows.tile([P, FTOT], I8)
            acc = rows.tile([P, 8], F32)

            wa = rows.tile([P, FTOT], F32)
            ha = rows.tile([P, FTOT], F32)

            # ---------- detection (iou/argmax) stage ----------
            nc.sync.dma_start(regt[:], reg3[:, :, :])
            regv = regt[:].rearrange("p (f c) -> p f c", c=4)
            nc.scalar.activation(ax1[:], regv[:, :, 0:1].squeeze(2), AF.Copy)
            nc.scalar.activation(ay1[:], regv[:, :, 1:2].squeeze(2), AF.Copy)
            nc.scalar.activation(ax2[:], regv[:, :, 2:3].squeeze(2), AF.Copy)
            nc.scalar.activation(ay2[:], regv[:, :, 3:4].squeeze(2), AF.Copy)
            nc.vector.memset(state[:], -3.0e38)  # float max-reduce over encodings
            nc.vector.tensor_tensor(wa[:], ax2[:], ax1[:], ALU.subtract)
            nc.vector.tensor_tensor(ha[:], ay2[:], ay1[:], ALU.subtract)
            nc.vector.tensor_tensor(areaA[:], wa[:], ha[:], ALU.mult)

            for m in range(M):
                mxq = mtmp.tile([P, FTOT], F32, tag="mxq")
                wq = mtmp.tile([P, FTOT], F32, tag="wq")
                myq = mtmp.tile([P, FTOT], F32, tag="myq")
                hq = mtmp.tile([P, FTOT], F32, tag="hq")
                hc = mtmp.tile([P, FTOT], F32, tag="hc")
                i_ = mtmp.tile([P, FTOT], F32, tag="i")
                li = mtmp.tile([P, FTOT], F32, tag="li")
                lab = mtmp.tile([P, FTOT], F32, tag="lab")
                lg = mtmp.tile([P, FTOT], F32, tag="lg")
                geb = mtmp.tile([P, FTOT], I32, tag="geb")

                # x-arm (ACT-shifted): mxq = max(ax1-b1x,0) = max(ax1,b1x)-b1x
                nc.scalar.activation(mxq[:], ax1[:], AF.Relu, bias=tcol(6, m))
                nc.vector.scalar_tensor_tensor(
                    wq[:], ax2[:], tcol(2, m), mxq[:], ALU.min, ALU.subtract
                )  # = w + b1x
                # y-arm (ACT-shifted)
                nc.scalar.activation(myq[:], ay1[:], AF.Relu, bias=tcol(5, m))
                nc.vector.scalar_tensor_tensor(
                    hq[:], ay2[:], tcol(3, m), myq[:], ALU.min, ALU.subtract
                )  # = h + b1y
                # hc = max(h, 0) = Relu(hq - b1y)
                nc.scalar.activation(hc[:], hq[:], AF.Relu, bias=tcol(5, m))
                # i = (wq - b1x) * hc = w * max(h,0); <=0 when no overlap
                nc.vector.scalar_tensor_tensor(
                    i_[:], wq[:], tcol(0, m), hc[:], ALU.subtract, ALU.mult
                )
                # log-domain ratio: ln(i) - ln(areaA + areaB_m)
                # Ln(<=0) -> NaN/-Inf; encoded bits become NaN patterns which
                # the DVE max suppresses (NaN-suppressing min/max), so bad
                # candidates drop out without any clamp.
                nc.scalar.activation(li[:], i_[:], AF.Ln)
                nc.scalar.activation(lab[:], areaA[:], AF.Ln, bias=tcol(4, m))
                nc.vector.tensor_tensor(lg[:], li[:], lab[:], ALU.subtract)
                # encode (bits & ~31) | m ; running float max
                nc.vector.tensor_scalar(
                    geb[:], lg[:].bitcast(I32), -32, m, ALU.bitwise_and, ALU.bitwise_or
                )
                nc.vector.tensor_tensor(
                    state[:], state[:], geb[:].bitcast(F32), ALU.max
                )

            # decode
            nc.vector.tensor_scalar(
                gbits[:], state[:].bitcast(I32), -32, None, ALU.bitwise_and
            )
            gmaxf = gbits[:].bitcast(F32)
            nc.vector.tensor_scalar(
                posm[:], gmaxf, THR_POS, None, ALU.is_ge, ALU.add,
                accum_out=acc[:, 0:1],
            )
            nc.vector.tensor_scalar(
                negm[:], gmaxf, THR_NEG, None, ALU.is_lt, ALU.add,
                accum_out=acc[:, 1:2],
            )
            nc.vector.tensor_scalar(
                idx32[:], state[:].bitcast(I32), 31, None, ALU.bitwise_and
            )
            nc.scalar.activation(idxf[:], idx32[:], AF.Copy)
            nc.vector.scalar_tensor_tensor(
                encf[:], posm[:], 32.0, idxf[:], ALU.mult, ALU.add
            )
            nc.vector.scalar_tensor_tensor(
                encf[:], negm[:], 64.0, encf[:], ALU.mult, ALU.add
            )
            nc.scalar.activation(enc8[:], encf[:], AF.Copy)
            nc.sync.dma_start(enc_d[:], enc8[:])

            # ---------- classification stage ----------
            for t in range(NTC):
                s = slice(t * FC, (t + 1) * FC)
                ct = clsbuf.tile([P, FC * C], F32, tag="cls")
                nc.sync.dma_start(ct[:], cls3[:, s, :])
                cv = ct[:].rearrange("p (f c) -> p f c", c=C)
                nc.scalar.activation(cls0[:, s], cv[:, :, 0:1].squeeze(2), AF.Copy)
                nc.scalar.activation(ct[:], ct[:], AF.Exp)  # in-place exp
                nc.vector.reduce_sum(sumexp[:, s], cv, AX.X)
            nc.scalar.activation(logS[:], sumexp[:], AF.Ln)

            # ---------- final partial sums ----------
            nc.vector.scalar_tensor_tensor(
                dummy[:], posm[:], 1.0, logS[:], ALU.mult, ALU.mult,
                accum_out=acc[:, 2:3],
            )
            nc.vector.scalar_tensor_tensor(
                dummy[:], negm[:], 1.0, logS[:], ALU.mult, ALU.mult,
                accum_out=acc[:, 3:4],
            )
            nc.vector.scalar_tensor_tensor(
                dummy[:], negm[:], 1.0, cls0[:], ALU.mult, ALU.mult,
                accum_out=acc[:, 4:5],
            )
            nc.vector.memset(acc[:, 5:8], 0.0)
            nc.sync.dma_start(acc_d[:], acc[:])

    nc.compile()
    return nc


def get_program():
    if "nc" not in _CACHE:
        _CACHE["nc"] = _build_program()
    return _CACHE["nc"]


def make_inmaps(cls_output, reg_output, target_boxes):
    """Per-core input dicts. cls/reg must be float32 numpy [B,N,C]/[B,N,4]."""
    in_maps = []
    for b in range(len(target_boxes)):
        tb = np.asarray(target_boxes[b], dtype=np.float32)
        area_b = (tb[:, 2] - tb[:, 0]) * (tb[:, 3] - tb[:, 1])
        tcon = np.empty((7, M), dtype=np.float32)
        tcon[0] = tb[:, 0]   # b1x
        tcon[1] = tb[:, 1]   # b1y
        tcon[2] = tb[:, 2]   # b2x
        tcon[3] = tb[:, 3]   # b2y
        tcon[4] = area_b
        tcon[5] = -tb[:, 1]  # -b1y (ACT relu bias)
        tcon[6] = -tb[:, 0]  # -b1x (ACT relu bias)
        tcon_rep = np.broadcast_to(tcon.reshape(1, 7 * M), (P, 7 * M)).copy()
        in_maps.append(
            {
                "cls": np.ascontiguousarray(cls_output[b], dtype=np.float32),
                "reg": np.ascontiguousarray(reg_output[b], dtype=np.float32),
                "tcon": tcon_rep,
            }
        )
    return in_maps


def host_combine(results, cls_output, reg_output, target_boxes, target_labels):
    """Combine per-core (acc, enc) into the reference's scalar loss."""
    nb = len(target_boxes)
    cp = np.zeros(nb)
    cn = np.zeros(nb)
    rl = np.zeros(nb)
    has_p = np.zeros(nb, dtype=bool)
    has_n = np.zeros(nb, dtype=bool)
    for b in range(nb):
        acc = results[b]["acc"].astype(np.float64).sum(axis=0)  # [8]
        enc = results[b]["enc"].reshape(-1).astype(np.int16)  # [N] anchor order
        enc = np.where(enc < 0, enc + 256, enc)  # int8 -> uint8 semantics safety
        idx = (enc & 31).astype(np.int64)
        pos = (enc & 32) != 0
        neg = (enc & 64) != 0
        npos = float(pos.sum())
        nneg = float(neg.sum())
        sum_pos_logS, sum_neg_logS, sum_neg_cls0 = acc[2], acc[3], acc[4]

        cls_b = np.asarray(cls_output[b], dtype=np.float64)
        reg_b = np.asarray(reg_output[b], dtype=np.float64)
        tb = np.asarray(target_boxes[b], dtype=np.float64)
        tl = np.asarray(target_labels[b]).astype(np.int64)

        pidx = np.nonzero(pos)[0]
        match = idx[pidx]
        sum_pos_cls = float(cls_b[pidx, tl[match]].sum())
        d = reg_b[pidx] - tb[match]
        ad = np.abs(d)
        sl1 = np.where(ad < 1.0, 0.5 * d * d, ad - 0.5).mean(axis=1)
        rl[b] = float(sl1.sum()) / max(npos, 1.0)

        cp[b] = (sum_pos_logS - sum_pos_cls) / max(npos, 1.0)
        cn[b] = (sum_neg_logS - sum_neg_cls0) / max(nneg, 1.0)
        has_p[b] = npos > 0
        has_n[b] = nneg > 0

    cls_vals = np.concatenate([cp, cn])
    cls_valid = np.concatenate([has_p, has_n]).astype(np.float64)
    n_cls = cls_valid.sum()
    cls_mean = (cls_vals * cls_valid).sum() / max(n_cls, 1.0)
    has_pf = has_p.astype(np.float64)
    n_reg = has_pf.sum()
    reg_mean = (rl * has_pf).sum() / max(n_reg, 1.0)
    total = (cls_mean if n_cls > 0 else 0.0) + (reg_mean if n_reg > 0 else 0.0)
    return np.float32(total)


def kernel(cls_output, reg_output, target_boxes, target_labels, _trace=False):
    cls_output = np.asarray(cls_output)
    reg_output = np.asarray(reg_output)
    target_boxes = np.asarray(target_boxes)
    target_labels = np.asarray(target_labels)
    nc = get_program()
    in_maps = make_inmaps(cls_output, reg_output, target_boxes)
    res = run_bass_kernel_spmd(nc, in_maps, list(range(B)), trace=_trace)
    total = host_combine(
        res.results, cls_output, reg_output, target_boxes, target_labels
    )
    if _trace:
        _CACHE["last_result"] = res
    return total


# revision 21
# speedup vs baseline: 1.0030x; 1.0030x over previous
"""Trainium2 Bass kernel for nn_DetectionLoss (B=8, N=131072, C=21, M=32).

Strategy (data-parallel over batch, one image per NeuronCore):
  Device (per core, SPMD over 8 cores):
    - classification side: exp -> group-sum over C=21 -> log  => logS per anchor
    - detection side: for each of M=32 targets, compute intersection i with all
      anchors via fused tensor_scalar / scalar_tensor_tensor ops; compare the
      iou-equivalent ratio r = i/(areaA+areaB) in LOG domain
      (ln(i) - ln(areaA+areaB), ACT engine Ln; ACT Reciprocal is banned).
      Running argmax over m is a float MAX over bitcast(log-ratio) with the
      target index packed into the 5 low mantissa bits (log-ratio < 0 always;
      or-ing index bits grows the magnitude of a negative float, so ties break
      toward the smallest m, matching jnp.argmax).
      iou >= 0.5  <=>  r >= 1/3 ;  iou < 0.3  <=>  r < 3/13 (monotone map).
    - work is split across DVE and ACT to balance engine occupancy (the
      Pool/GpSimd engine rejects generic elementwise opcodes on this ISA).
    - outputs: per-anchor int8 encoding (argmax idx | pos<<5 | neg<<6) and
      per-partition partial sums (npos, nneg, sum pos*logS, sum neg*logS,
      sum neg*cls0).
  Host: O(npos)-sized gathers for the matched-class CE term and the
  smooth-L1 regression term, then the reference's scalar combine.
"""

import sys
import numpy as np

sys.path.insert(0, "/opt/trn_rl_repo")

import concourse.bass as bass  # noqa: E402
import concourse.tile as tile  # noqa: E402
from concourse import bacc, mybir  # noqa: E402
from concourse.bass_utils import run_bass_kernel_spmd  # noqa: E402

F32 = mybir.dt.float32
I32 = mybir.dt.int32
I8 = mybir.dt.int8
ALU = mybir.AluOpType
AF = mybir.ActivationFunctionType
AX = mybir.AxisListType

B, N, C, M = 8, 131072, 21, 32
P = 128
FTOT = N // P          # 1024 anchors per partition
FC = 128               # cls-stage tile free size
NTC = FTOT // FC       # cls tiles

THR_POS = float(np.float32(np.log(1.0 / 3.0)))    # iou>=0.5  <=> logratio >= ln(1/3)
THR_NEG = float(np.float32(np.log(3.0 / 13.0)))   # iou<0.3   <=> logratio <  ln(3/13)

# NOTE: the Pool/GpSimd engine rejects TensorTensor/TensorScalar on this ISA
# (walrus NCC_IXCG966), so all elementwise work lives on DVE + ACT.

_CACHE = {}


def _patch_act_tables():
    """Force exp+ln onto the single combined table set (kills table thrash)."""
    from concourse import hw_specs

    orig = hw_specs.get_activation_tables
    if getattr(bacc.get_activation_tables, "_patched", False):
        return

    def patched(arch):
        t = orig(arch)
        exp = mybir.ActivationFunctionType.Exp
        ln = mybir.ActivationFunctionType.Ln
        for name, fns in t.items():
            if name != "natural_log_exp_and_others":
                fns.discard(exp)
                fns.discard(ln)
        return t

    patched._patched = True
    bacc.get_activation_tables = patched


def _build_program():
    _patch_act_tables()
    nc = bacc.Bacc("TRN2", target_bir_lowering=False, debug=False)

    cls_d = nc.dram_tensor("cls", [N, C], F32, kind="ExternalInput").ap()
    reg_d = nc.dram_tensor("reg", [N, 4], F32, kind="ExternalInput").ap()
    tcon_d = nc.dram_tensor("tcon", [P, 7 * M], F32, kind="ExternalInput").ap()
    acc_d = nc.dram_tensor("acc", [P, 8], F32, kind="ExternalOutput").ap()
    enc_d = nc.dram_tensor("enc", [P, FTOT], I8, kind="ExternalOutput").ap()

    cls3 = cls_d.rearrange("(p f) c -> p f c", p=P)   # [128, 1024, 21]
    reg3 = reg_d.rearrange("(p f) c -> p f c", p=P)   # [128, 1024, 4]

    with tile.TileContext(nc) as tc:
        with (
            tc.tile_pool(name="const", bufs=1) as constp,
            tc.tile_pool(name="rows", bufs=1) as rows,
            tc.tile_pool(name="clsbuf", bufs=3) as clsbuf,
            tc.tile_pool(name="mtmp", bufs=2) as mtmp,
        ):
            tcon = constp.tile([P, 7 * M], F32)
            nc.sync.dma_start(tcon[:], tcon_d[:])

            def tcol(c, m):
                # blocks: 0=b1x 1=b1y 2=b2x 3=b2y 4=area_b 5=-b1y 6=-b1x
                return tcon[:, c * M + m : c * M + m + 1]

            # persistent full-row tensors
            regt = rows.tile([P, FTOT * 4], F32)
            ax1 = rows.tile([P, FTOT], F32)
            ay1 = rows.tile([P, FTOT], F32)
            ax2 = rows.tile([P, FTOT], F32)
            ay2 = rows.tile([P, FTOT], F32)
            areaA = rows.tile([P, FTOT], F32)
            state = rows.tile([P, FTOT], F32)
            sumexp = rows.tile([P, FTOT], F32)
            logS = rows.tile([P, FTOT], F32)
            cls0 = rows.tile([P, FTOT], F32)
            posm = rows.tile([P, FTOT], F32)
            negm = rows.tile([P, FTOT], F32)
            idxf = rows.tile([P, FTOT], F32)
            encf = rows.tile([P, FTOT], F32)
            dummy = rows.tile([P, FTOT], F32)
            gbits = rows.tile([P, FTOT], I32)
            idx32 = rows.tile([P, FTOT], I32)
            enc8 = rows.tile([P, FTOT], I8)
            acc = rows.tile([P, 8], F32)

            wa = rows.tile([P, FTOT], F32)
            ha = rows.tile([P, FTOT], F32)

            # ---------- detection (iou/argmax) stage ----------
            nc.sync.dma_start(regt[:], reg3[:, :, :])
            regv = regt[:].rearrange("p (f c) -> p f c", c=4)
            nc.scalar.activation(ax1[:], regv[:, :, 0:1].squeeze(2), AF.Copy)
            nc.scalar.activation(ay1[:], regv[:, :, 1:2].squeeze(2), AF.Copy)
            nc.scalar.activation(ax2[:], regv[:, :, 2:3].squeeze(2), AF.Copy)
            nc.scalar.activation(ay2[:], regv[:, :, 3:4].squeeze(2), AF.Copy)
            nc.vector.memset(state[:], -3.0e38)  # float max-reduce over encodings
            nc.vector.tensor_tensor(wa[:], ax2[:], ax1[:], ALU.subtract)
            nc.vector.tensor_tensor(ha[:], ay2[:], ay1[:], ALU.subtract)
            nc.vector.tensor_tensor(areaA[:], wa[:], ha[:], ALU.mult)

            for m in range(M):
                mxq = mtmp.tile([P, FTOT], F32, tag="mxq")
                wq = mtmp.tile([P, FTOT], F32, tag="wq")
                myq = mtmp.tile([P, FTOT], F32, tag="myq")
                hq = mtmp.tile([P, FTOT], F32, tag="hq")
                hc = mtmp.tile([P, FTOT], F32, tag="hc")
                i_ = mtmp.tile([P, FTOT], F32, tag="i")
                li = mtmp.tile([P, FTOT], F32, tag="li")
                lab = mtmp.tile([P, FTOT], F32, tag="lab")
                lg = mtmp.tile([P, FTOT], F32, tag="lg")
                geb = mtmp.tile([P, FTOT], I32, tag="geb")

                # x-arm (ACT-shifted): mxq = max(ax1-b1x,0) = max(ax1,b1x)-b1x
                nc.scalar.activation(mxq[:], ax1[:], AF.Relu, bias=tcol(6, m))
                nc.vector.scalar_tensor_tensor(
                    wq[:], ax2[:], tcol(2, m), mxq[:], ALU.min, ALU.subtract
                )  # = w + b1x
                # y-arm (ACT-shifted)
                nc.scalar.activation(myq[:], ay1[:], AF.Relu, bias=tcol(5, m))
                nc.vector.scalar_tensor_tensor(
                    hq[:], ay2[:], tcol(3, m), myq[:], ALU.min, ALU.subtract
                )  # = h + b1y
                # hc = max(h, 0) = Relu(hq - b1y)
                nc.scalar.activation(hc[:], hq[:], AF.Relu, bias=tcol(5, m))
                # i = (wq - b1x) * hc = w * max(h,0); <=0 when no overlap
                nc.vector.scalar_tensor_tensor(
                    i_[:], wq[:], tcol(0, m), hc[:], ALU.subtract, ALU.mult
                )
                # log-domain ratio: ln(i) - ln(areaA + areaB_m)
                # Ln(<=0) -> NaN/-Inf; encoded bits become NaN patterns which
                # the DVE max suppresses (NaN-suppressing min/max), so bad
                # candidates drop out without any clamp.
                nc.scalar.activation(li[:], i_[:], AF.Ln)
                nc.scalar.activation(lab[:], areaA[:], AF.Ln, bias=tcol(4, m))
                nc.vector.tensor_tensor(lg[:], li[:], lab[:], ALU.subtract)
                # encode (bits & ~31) | m ; running float max
                nc.vector.tensor_scalar(
                    geb[:], lg[:].bitcast(I32), -32, m, ALU.bitwise_and, ALU.bitwise_or
                )
                nc.vector.tensor_tensor(
                    state[:], state[:], geb[:].bitcast(F32), ALU.max
                )

            # decode
            nc.vector.tensor_scalar(
                gbits[:], state[:].bitcast(I32), -32, None, ALU.bitwise_and
            )
            gmaxf = gbits[:].bitcast(F32)
            nc.vector.tensor_scalar(
                posm[:], gmaxf, THR_POS, None, ALU.is_ge, ALU.add,
                accum_out=acc[:, 0:1],
            )
            nc.vector.tensor_scalar(
                negm[:], gmaxf, THR_NEG, None, ALU.is_lt, ALU.add,
                accum_out=acc[:, 1:2],
            )
            nc.vector.tensor_scalar(
                idx32[:], state[:].bitcast(I32), 31, None, ALU.bitwise_and
            )
            nc.scalar.activation(idxf[:], idx32[:], AF.Copy)
            nc.vector.scalar_tensor_tensor(
                encf[:], posm[:], 32.0, idxf[:], ALU.mult, ALU.add
            )
            nc.vector.scalar_tensor_tensor(
                encf[:], negm[:], 64.0, encf[:], ALU.mult, ALU.add
            )
            nc.scalar.activation(enc8[:], encf[:], AF.Copy)
            nc.sync.dma_start(enc_d[:], enc8[:])

            # ---------- classification stage ----------
            for t in range(NTC):
                s = slice(t * FC, (t + 1) * FC)
                ct = clsbuf.tile([P, FC * C], F32, tag="cls")
                nc.sync.dma_start(ct[:], cls3[:, s, :])
                cv = ct[:].rearrange("p (f c) -> p f c", c=C)
                nc.scalar.activation(cls0[:, s], cv[:, :, 0:1].squeeze(2), AF.Copy)
                nc.scalar.activation(ct[:], ct[:], AF.Exp)  # in-place exp
                nc.vector.reduce_sum(sumexp[:, s], cv, AX.X)
            nc.scalar.activation(logS[:], sumexp[:], AF.Ln)

            # ---------- final partial sums ----------
            nc.vector.scalar_tensor_tensor(
                dummy[:], posm[:], 1.0, logS[:], ALU.mult, ALU.mult,
                accum_out=acc[:, 2:3],
            )
            nc.vector.scalar_tensor_tensor(
                dummy[:], negm[:], 1.0, logS[:], ALU.mult, ALU.mult,
                accum_out=acc[:, 3:4],
            )
            nc.vector.scalar_tensor_tensor(
                dummy[:], negm[:], 1.0, cls0[:], ALU.mult, ALU.mult,
                accum_out=acc[:, 4:5],
            )
            nc.vector.memset(acc[:, 5:8], 0.0)
            nc.sync.dma_start(acc_d[:], acc[:])

    nc.compile()
    return nc


def get_program():
    if "nc" not in _CACHE:
        _CACHE["nc"] = _build_program()
    return _CACHE["nc"]


def make_inmaps(cls_output, reg_output, target_boxes):
    """Per-core input dicts. cls/reg must be float32 numpy [B,N,C]/[B,N,4]."""
    in_maps = []
    for b in range(len(target_boxes)):
        tb = np.asarray(target_boxes[b], dtype=np.float32)
        area_b = (tb[:, 2] - tb[:, 0]) * (tb[:, 3] - tb[:, 1])
        tcon = np.empty((7, M), dtype=np.float32)
        tcon[0] = tb[:, 0]   # b1x
        tcon[1] = tb[:, 1]   # b1y
        tcon[2] = tb[:, 2]   # b2x
        tcon[3] = tb[:, 3]   # b2y
        tcon[4] = area_b
        tcon[5] = -tb[:, 1]  # -b1y (ACT relu bias)
        tcon[6] = -tb[:, 0]  # -b1x (ACT relu bias)
        tcon_rep = np.broadcast_to(tcon.reshape(1, 7 * M), (P, 7 * M)).copy()
        in_maps.append(
            {
                "cls": np.ascontiguousarray(cls_output[b], dtype=np.float32),
                "reg": np.ascontiguousarray(reg_output[b], dtype=np.float32),
                "tcon": tcon_rep,
            }
        )
    return in_maps


def host_combine(results, cls_output, reg_output, target_boxes, target_labels):
    """Combine per-core (acc, enc) into the reference's scalar loss."""
    nb = len(target_boxes)
    cp = np.zeros(nb)
    cn = np.zeros(nb)
    rl = np.zeros(nb)
    has_p = np.zeros(nb, dtype=bool)
    has_n = np.zeros(nb, dtype=bool)
    for b in range(nb):
        acc = results[b]["acc"].astype(np.float64).sum(axis=0)  # [8]
        enc = results[b]["enc"].reshape(-1).astype(np.int16)  # [N] anchor order
        enc = np.where(enc < 0, enc + 256, enc)  # int8 -> uint8 semantics safety
        idx = (enc & 31).astype(np.int64)
        pos = (enc & 32) != 0
        neg = (enc & 64) != 0
        npos = float(pos.sum())
        nneg = float(neg.sum())
        sum_pos_logS, sum_neg_logS, sum_neg_cls0 = acc[2], acc[3], acc[4]

        cls_b = np.asarray(cls_output[b], dtype=np.float64)
        reg_b = np.asarray(reg_output[b], dtype=np.float64)
        tb = np.asarray(target_boxes[b], dtype=np.float64)
        tl = np.asarray(target_labels[b]).astype(np.int64)

        pidx = np.nonzero(pos)[0]
        match = idx[pidx]
        sum_pos_cls = float(cls_b[pidx, tl[match]].sum())
        d = reg_b[pidx] - tb[match]
        ad = np.abs(d)
        sl1 = np.where(ad < 1.0, 0.5 * d * d, ad - 0.5).mean(axis=1)
        rl[b] = float(sl1.sum()) / max(npos, 1.0)

        cp[b] = (sum_pos_logS - sum_pos_cls) / max(npos, 1.0)
        cn[b] = (sum_neg_logS - sum_neg_cls0) / max(nneg, 1.0)
        has_p[b] = npos > 0
        has_n[b] = nneg > 0

    cls_vals = np.concatenate([cp, cn])
    cls_valid = np.concatenate([has_p, has_n]).astype(np.float64)
    n_cls = cls_valid.sum()
    cls_mean = (cls_vals * cls_valid).sum() / max(n_cls, 1.0)
    has_pf = has_p.astype(np.float64)
    n_reg = has_pf.sum()
    reg_mean = (rl * has_pf).sum() / max(n_reg, 1.0)
    total = (cls_mean if n_cls > 0 else 0.0) + (reg_mean if n_reg > 0 else 0.0)
    return np.float32(total)


def kernel(cls_output, reg_output, target_boxes, target_labels, _trace=False):
    cls_output = np.asarray(cls_output)
    reg_output = np.asarray(reg_output)
    target_boxes = np.asarray(target_boxes)
    target_labels = np.asarray(target_labels)
    nc = get_program()
    in_maps = make_inmaps(cls_output, reg_output, target_boxes)
    res = run_bass_kernel_spmd(nc, in_maps, list(range(B)), trace=_trace)
    total = host_combine(
        res.results, cls_output, reg_output, target_boxes, target_labels
    )
    if _trace:
        _CACHE["last_result"] = res
    return total
